# revision 13
# baseline (speedup 1.0000x reference)
"""Trainium2 Bass kernel for nn_BertGNNGru (attention-gated GRU scan).

Strategy (data-parallel over batch, 8 cores x 128 rows; v2):
  - Fold the attention gate algebraically into the GRU weight matrices:
      inputgate = sigmoid([i_i|h_i] @ Wa.T + ba)
                = sigmoid(x@(Wa_i@Wx_i).T + h@(Wa_h@Wh_i).T + const)
    so the whole step becomes one 768-row "projection" from x and one from h.
  - All tensors live in the transposed [feature-on-partitions, batch-on-free]
    layout for the whole kernel.  x is pre-transposed (and bf16-cast) on the
    host; y is stored transposed and un-transposed on the host.  This removes
    every on-device transpose and PSUM->SBUF copy of the output path.
  - r and z share one merged sigmoid (their pre-activations live in one PSUM
    bank), biases ride in PSUM via a K=4 indicator matmul.
  - The per-core batch of 128 is split into CHAINS independent recurrences
    (64 rows each) that are software-pipelined against each other: while
    chain A runs its activation/elementwise tail, chain B uses the PE/ACT.
  - Off-critical elementwise work (1-z, z*h_prev) runs on the otherwise idle
    GPSIMD engine.
"""

import os
from contextlib import ExitStack

import numpy as np
import ml_dtypes

import concourse.bass as bass
import concourse.tile as tile
from concourse import bacc, mybir
from concourse import bass_utils

F32 = mybir.dt.float32
BF16 = mybir.dt.bfloat16
FP8 = mybir.dt.float8e4

B, T_FULL, D, H = 1024, 512, 256, 256
NCORES = 8
BS = B // NCORES          # 128 batch rows per core
G3 = 3 * H                # 768 folded projection rows: [r | a | n]
ADD, SUB, MUL = mybir.AluOpType.add, mybir.AluOpType.subtract, mybir.AluOpType.mult
SIG, TANH = mybir.ActivationFunctionType.Sigmoid, mybir.ActivationFunctionType.Tanh

CHAINS = int(os.environ.get("GRU_CHAINS", "2"))
CW = BS // CHAINS         # batch columns per chain
LA = int(os.environ.get("GRU_LA", "2"))       # x-phase lookahead (steps)
XGRP = int(os.environ.get("GRU_XGRP", "8"))   # steps per x load
YGRP = int(os.environ.get("GRU_YGRP", "8"))   # steps per y store
XF8 = os.environ.get("GRU_XF8", "0") == "1"   # fp8 DoubleRow x-side matmuls


def _emit(ctx: ExitStack, tc: tile.TileContext, x_d, wpx_d, wph_d, bias_d, y_d, T):
    nc = tc.nc

    # ---------------- pools ----------------
    wpool = ctx.enter_context(tc.tile_pool(name="w", bufs=1))
    xpool = ctx.enter_context(tc.tile_pool(name="x", bufs=int(os.environ.get("GRU_XBUFS", "3"))))
    psA = ctx.enter_context(tc.tile_pool(name="psA", bufs=int(os.environ.get("GRU_PSA", "3")), space="PSUM"))
    psB = ctx.enter_context(tc.tile_pool(name="psB", bufs=int(os.environ.get("GRU_PSB", "3")), space="PSUM"))
    ew = ctx.enter_context(tc.tile_pool(name="ew", bufs=int(os.environ.get("GRU_EWBUFS", "3"))))
    gp = ctx.enter_context(tc.tile_pool(name="gp", bufs=int(os.environ.get("GRU_GPBUFS", "3"))))
    yrow_pool = ctx.enter_context(tc.tile_pool(name="yrow", bufs=int(os.environ.get("GRU_YRBUFS", "2"))))

    # ---------------- constants ----------------
    wpx_sb = []
    wph_sb = []
    if XF8:
        wx = wpool.tile([128, 2, G3], FP8, tag="wpx0")
        nc.sync.dma_start(wx[:], wpx_d[0])
        wpx_sb.append(wx)
    else:
        for k in range(2):
            wx = wpool.tile([128, G3], BF16, tag=f"wpx{k}")
            nc.sync.dma_start(wx[:], wpx_d[k])
            wpx_sb.append(wx)
    for k in range(2):
        wh = wpool.tile([128, G3], BF16, tag=f"wph{k}")
        nc.sync.dma_start(wh[:], wph_d[k])
        wph_sb.append(wh)
    bwA_sb = wpool.tile([4, 128], BF16, tag="bwA")
    nc.sync.dma_start(bwA_sb[:], bias_d[0, :, :128])
    bwB_sb = wpool.tile([4, 128], BF16, tag="bwB")
    nc.sync.dma_start(bwB_sb[:], bias_d[1, :, :128])
    ind_sb = wpool.tile([4, 512], BF16, tag="ind")
    nc.sync.dma_start(ind_sb[:], bias_d[2, :, :512])

    # ---------------- per-step state ----------------
    pA = {}     # step -> PSUM [128, 512] = [pre_r0 | pre_r1 | pre_a0 | pre_a1]
    pB = {}     # step -> PSUM [128, 512] = [ghn0 | ghn1 | pxn0 | pxn1]
    xsb = {}    # x-group -> SBUF tile
    yrow = {}   # y-group -> SBUF [128, YGRP*2*BS] bf16 (per-step slots of hyT)
    rz_t = {}   # (step, chain) -> rz tile
    n_t = {}    # (step, chain) -> n tile
    zp_t = {}   # (step, chain) -> 1-z tile
    t3_t = {}   # (step, chain) -> z*h tile

    SLOT = 2 * BS  # columns per step slot in yrow: [blk0 batch | blk1 batch]

    def hy_ap(t, c, k=None):
        """AP into yrow for chain c's hidden state at step t.
        k=None -> both feature blocks (strided); k=0/1 -> single block."""
        g, slot = divmod(t, YGRP)
        base = slot * SLOT
        if k is not None:
            return yrow[g][:, base + k * BS + c * CW : base + k * BS + c * CW + CW]
        if CHAINS == 1:
            return yrow[g][:, base : base + SLOT]
        slot_ap = yrow[g][:, base : base + SLOT]
        return slot_ap.rearrange("p (k ch b) -> p ch k b", k=2, ch=CHAINS)[:, c]

    def x_phase(t):
        """x load (per group) + x-side matmuls + bias deposit for step t."""
        g, s = divmod(t, XGRP)
        if s == 0:
            steps = min(XGRP, T - g * XGRP)
            xt = xpool.tile([128, steps, 2 * BS], FP8 if XF8 else BF16, tag="xsb", name="xsb")
            src = x_d[g * XGRP : g * XGRP + steps].rearrange("t p c -> p t c")
            nc.sync.dma_start(xt[:], src)
            xsb[g] = xt
        a = psA.tile([128, 512], F32, tag="pA", name="pA")
        b = psB.tile([128, 512], F32, tag="pB", name="pB")
        pA[t] = a
        pB[t] = b
        # K=4 indicator matmuls open each bank and deposit the four bias
        # vectors into their 128-column blocks.
        nc.tensor.matmul(a[:, :], bwA_sb[:], ind_sb[:], start=True, stop=False)
        nc.tensor.matmul(b[:, :], bwB_sb[:], ind_sb[:], start=True, stop=False)
        last_x = t == 0  # at t=0 there are no h-side matmuls
        if XF8:
            xk = xsb[g][:, s].rearrange("p (k n) -> p k n", k=2)
            for g4 in range(4):
                nc.tensor.matmul(
                    a[:, g4 * 128 : (g4 + 1) * 128],
                    wpx_sb[0][:, :, g4 * 128 : (g4 + 1) * 128],
                    xk,
                    start=False,
                    stop=(last_x and g4 == 3),
                    perf_mode=mybir.MatmulPerfMode.DoubleRow,
                )
            for gi in range(2):
                nc.tensor.matmul(
                    b[:, 256 + gi * 128 : 256 + (gi + 1) * 128],
                    wpx_sb[0][:, :, (4 + gi) * 128 : (5 + gi) * 128],
                    xk,
                    start=False,
                    stop=(last_x and gi == 1),
                    perf_mode=mybir.MatmulPerfMode.DoubleRow,
                )
        else:
            for k in range(2):
                xk = xsb[g][:, s, k * BS : (k + 1) * BS]
                for g4 in range(4):  # r0 r1 a0 a1
                    nc.tensor.matmul(
                        a[:, g4 * 128 : (g4 + 1) * 128],
                        wpx_sb[k][:, g4 * 128 : (g4 + 1) * 128],
                        xk,
                        start=False,
                        stop=(last_x and k == 1 and g4 == 3),
                    )
                for gi in range(2):  # n0 n1 -> pB cols 256:512
                    nc.tensor.matmul(
                        b[:, 256 + gi * 128 : 256 + (gi + 1) * 128],
                        wpx_sb[k][:, (4 + gi) * 128 : (5 + gi) * 128],
                        xk,
                        start=False,
                        stop=(last_x and k == 1 and gi == 1),
                    )

    def h_mm(t, c):
        """h-side matmuls for step t, chain c (t >= 1)."""
        a, b = pA[t], pB[t]
        last_c = c == CHAINS - 1
        # order: r0 r1 a0 a1 (-> rz asap), then ghn0 ghn1 (-> u)
        seq = (
            (0, a, 0, False), (1, a, 128, False),
            (2, a, 256, False), (3, a, 384, True),
            (4, b, 0, False), (5, b, 128, True),
        )
        for g6, bank, col, last_g in seq:
            for k in range(2):
                nc.tensor.matmul(
                    bank[:, col + c * CW : col + c * CW + CW],
                    wph_sb[k][:, g6 * 128 : (g6 + 1) * 128],
                    hy_ap(t - 1, c, k=k),
                    start=False,
                    stop=(last_g and last_c and k == 1),
                )

    def ew1(t, c):
        """rz sigmoid + u/t1 + n tanh + gpsimd helpers for (t, c)."""
        a, b = pA[t], pB[t]
        if CHAINS == 1:
            a_in = a[:, 0:512]
            ghn = b[:, 0:256]
            pxn = b[:, 256:512]
        else:
            a_in = a[:, 0:512].rearrange("p (g ch b) -> p ch g b", g=4, ch=CHAINS)[:, c]
            bb = b[:, 0:512].rearrange("p (g ch b) -> p ch g b", g=4, ch=CHAINS)
            ghn = bb[:, c, 0:2]
            pxn = bb[:, c, 2:4]
        rz = ew.tile([128, 4 * CW], BF16, tag=f"rz{c}", name="rz")
        nc.scalar.activation(rz[:], a_in, SIG)
        rz_t[(t, c)] = rz
        u = ew.tile([128, 2 * CW], BF16, tag=f"u{c}", name="u")
        nc.vector.tensor_tensor(u[:], rz[:, 0 : 2 * CW], ghn, MUL)
        t1 = ew.tile([128, 2 * CW], BF16, tag=f"t1{c}", name="t1")
        nc.vector.tensor_tensor(t1[:], u[:], pxn, ADD)
        n = ew.tile([128, 2 * CW], BF16, tag=f"n{c}", name="n")
        nc.scalar.activation(n[:], t1[:], TANH)
        n_t[(t, c)] = n
        # gpsimd (off critical path): zp = 1 - z ; t3 = z * h_prev
        zp = gp.tile([128, 2 * CW], BF16, tag=f"zp{c}", name="zp")
        nc.gpsimd.tensor_scalar(zp[:], rz[:, 2 * CW : 4 * CW], -1.0, 1.0, MUL, ADD)
        zp_t[(t, c)] = zp
        if t > 0:
            t3 = gp.tile([128, 2 * CW], BF16, tag=f"t3{c}", name="t3")
            nc.gpsimd.tensor_tensor(t3[:], rz[:, 2 * CW : 4 * CW], hy_ap(t - 1, c), MUL)
            t3_t[(t, c)] = t3

    def ew2(t, c):
        """w = n*zp ; hy = w + t3, written into the yrow slot for step t."""
        g, slot = divmod(t, YGRP)
        if slot == 0 and c == 0:
            steps = min(YGRP, T - g * YGRP)
            yrow[g] = yrow_pool.tile([128, steps * SLOT], BF16, tag="yrow", name="yrow")
        n = n_t.pop((t, c))
        zp = zp_t.pop((t, c))
        hy_dst = hy_ap(t, c)
        if t > 0:
            w = ew.tile([128, 2 * CW], BF16, tag=f"w{c}", name="w")
            nc.vector.tensor_tensor(w[:], n[:], zp[:], MUL)
            nc.vector.tensor_tensor(hy_dst, w[:], t3_t.pop((t, c))[:], ADD)
        else:
            nc.vector.tensor_tensor(hy_dst, n[:], zp[:], MUL)
        rz_t.pop((t, c), None)
        if c == CHAINS - 1:
            pA.pop(t, None)
            pB.pop(t, None)
            if (t + 1) % YGRP == 0 or t == T - 1:
                g0 = t // YGRP
                steps = min(YGRP, T - g0 * YGRP)
                dst = y_d[g0 * YGRP : g0 * YGRP + steps].rearrange("t p c -> p t c")
                src = yrow[g0][:].rearrange("p (t c) -> p t c", t=steps)
                nc.gpsimd.dma_start(dst, src)  # bf16 -> fp32 cast store

    # ---------------- main loop (software pipelined) ----------------
    # Per step t the emission order is:
    #   [ew2(t-1, A); h_mm(t, A); ew1(t, A)]; x_phase(t+LA);
    #   [ew2(t-1, B); h_mm(t, B); ew1(t, B)]
    for s in range(min(LA, T)):
        x_phase(s)
    for t in range(T + 1):
        for c in range(CHAINS):
            if t >= 1:
                ew2(t - 1, c)
            if 1 <= t < T:
                h_mm(t, c)
            if t < T:
                ew1(t, c)
            if c == 0 and LA <= t + LA < T:
                x_phase(t + LA)


def _build(T):
    nc = bacc.Bacc(
        "TRN2",
        target_bir_lowering=False,
        debug=False,
        num_devices=NCORES,
    )
    if XF8:
        x_d = nc.dram_tensor("x", [T, 128, 2 * BS], FP8, kind="ExternalInput").ap()
        wpx_d = nc.dram_tensor("wpx", [1, 128, 2, G3], FP8, kind="ExternalInput").ap()
    else:
        x_d = nc.dram_tensor("x", [T, 128, 2 * BS], BF16, kind="ExternalInput").ap()
        wpx_d = nc.dram_tensor("wpx", [2, 128, G3], BF16, kind="ExternalInput").ap()
    wph_d = nc.dram_tensor("wph", [2, 128, G3], BF16, kind="ExternalInput").ap()
    bias_d = nc.dram_tensor("bias", [3, 4, 512], BF16, kind="ExternalInput").ap()
    y_d = nc.dram_tensor("y", [T, 128, 2 * BS], F32, kind="ExternalOutput").ap()
    with tile.TileContext(nc) as tc:
        with ExitStack() as ctx:
            # emit LA x-phases up front so the recurrence starts with data
            _emit(ctx, tc, x_d, wpx_d, wph_d, bias_d, y_d, T)
    nc.compile()
    return nc


def _host_prep(Wx, bx, Wh, bh, Wa, ba):
    """Fold the attention gate into 768-row projection matrices (fp32 math)."""
    Wx_r, Wx_i, Wx_n = Wx[:H], Wx[H : 2 * H], Wx[2 * H :]
    Wh_r, Wh_i, Wh_n = Wh[:H], Wh[H : 2 * H], Wh[2 * H :]
    Wa_i, Wa_h = Wa[:, :H], Wa[:, H:]
    Wxa = Wa_i @ Wx_i
    Wha = Wa_h @ Wh_i
    bias_r = bx[:H] + bh[:H]
    bias_a = ba + Wa_i @ bx[H : 2 * H] + Wa_h @ bh[H : 2 * H]
    bh_n = bh[2 * H :]
    bx_n = bx[2 * H :]
    Wpx = np.concatenate([Wx_r, Wxa, Wx_n], axis=0)  # [768, 256]
    Wph = np.concatenate([Wh_r, Wha, Wh_n], axis=0)  # [768, 256]
    if XF8:
        # [128, 2, G3]: partition p, k-tile i, feature m = Wpx[m, i*128+p]
        wpx = np.ascontiguousarray(
            Wpx.T.reshape(2, 128, G3).transpose(1, 0, 2)
        ).astype(ml_dtypes.float8_e4m3)[None]
    else:
        wpx = np.ascontiguousarray(
            Wpx.T.reshape(2, 128, G3).astype(ml_dtypes.bfloat16)
        )
    wph = np.ascontiguousarray(
        Wph.T.reshape(2, 128, G3).astype(ml_dtypes.bfloat16)
    )
    bias = np.zeros((3, 4, 512), dtype=ml_dtypes.bfloat16)
    bias[0, 0, :128] = bias_r[:128]
    bias[0, 1, :128] = bias_r[128:]
    bias[0, 2, :128] = bias_a[:128]
    bias[0, 3, :128] = bias_a[128:]
    bias[1, 0, :128] = bh_n[:128]
    bias[1, 1, :128] = bh_n[128:]
    bias[1, 2, :128] = bx_n[:128]
    bias[1, 3, :128] = bx_n[128:]
    for k in range(4):
        bias[2, k, k * 128 : (k + 1) * 128] = 1.0
    return wpx, wph, bias


def kernel(x, Wx, bx, Wh, bh, Wa, ba):
    x = np.asarray(x, dtype=np.float32)
    Wx, bx, Wh, bh, Wa, ba = (
        np.asarray(a, dtype=np.float32) for a in (Wx, bx, Wh, bh, Wa, ba)
    )
    Bx, T = x.shape[0], x.shape[1]
    ncores = NCORES
    bs = Bx // ncores
    assert bs == BS
    wpx, wph, bias = _host_prep(Wx, bx, Wh, bh, Wa, ba)
    nc = _build(T)
    xdt = ml_dtypes.float8_e4m3 if XF8 else ml_dtypes.bfloat16
    in_maps = []
    for c in range(ncores):
        xs = x[c * BS : (c + 1) * BS]  # [BS, T, D]
        # [T, 128, 2*BS]: (t, p, k*BS+b) = x[b, t, k*128+p]
        xt = np.ascontiguousarray(
            xs.transpose(1, 2, 0).reshape(T, 2, 128, BS).transpose(0, 2, 1, 3)
            .reshape(T, 128, 2 * BS)
        ).astype(xdt)
        in_maps.append({"x": xt, "wpx": wpx, "wph": wph, "bias": bias})
    res = bass_utils.run_bass_kernel_spmd(
        nc,
        in_maps,
        core_ids=list(range(ncores)),
        trace=bool(int(os.environ.get("GRU_TRACE", "0"))),
    )
    global LAST_RESULTS
    LAST_RESULTS = res
    out = np.empty((Bx, T, H), dtype=np.float32)
    for c in range(ncores):
        ys = res.results[c]["y"]  # [T, 128, 2*BS]
        # y[b, t, blk*128 + p] = ys[t, p, blk*BS + b]
        out[c * BS : (c + 1) * BS] = (
            ys.reshape(T, 128, 2, BS).transpose(3, 0, 2, 1).reshape(BS, T, H)
        )
    return out


LAST_RESULTS = None


if __name__ == "__main__":
    # smoke test with random data at reduced T
    Tt = int(os.environ.get("GRU_T", "16"))
    rng = np.random.default_rng(0)
    std = 1.0 / np.sqrt(H)
    x = rng.standard_normal((B, Tt, D), dtype=np.float32)
    u = lambda shape: rng.uniform(-std, std, shape).astype(np.float32)
    args = dict(
        x=x, Wx=u((G3, D)), bx=u((G3,)), Wh=u((G3, H)), bh=u((G3,)),
        Wa=u((H, 2 * H)), ba=u((H,)),
    )
    out = kernel(**args)
    # numpy reference
    def ref(x, Wx, bx, Wh, bh, Wa, ba):
        h = np.zeros((B, H), np.float32)
        outs = np.empty((B, Tt, H), np.float32)
        for t in range(Tt):
            gx = x[:, t] @ Wx.T + bx
            gh = h @ Wh.T + bh
            r = 1 / (1 + np.exp(-(gx[:, :H] + gh[:, :H])))
            att = np.concatenate([gx[:, H : 2 * H], gh[:, H : 2 * H]], 1)
            z = 1 / (1 + np.exp(-(att @ Wa.T + ba)))
            n = np.tanh(gx[:, 2 * H :] + r * gh[:, 2 * H :])
            hy = n + z * (h - n)
            h = hy
            outs[:, t] = hy
        return outs

    expected = ref(**args)
    err = np.linalg.norm(out - expected) / np.linalg.norm(expected)
    print("rel_l2 =", err)
    print("maxabs =", np.abs(out - expected).max(), "ref absmax", np.abs(expected).max())
